# revision 2
# baseline (speedup 1.0000x reference)
"""Embedding gather-sum kernel for Trainium2 (8 NeuronCores, SPMD).

Problem (nn_UserLinearUpscaler):
    out[b, s, :] = sum_k W[:, ids[b, s, k]] + bias
    B=1024, S=50, K=20, E=64, V=100000, f32 weights, integer ids.

Sharding: data-parallel over batch — each of the 8 cores handles 128 batch
rows (6400 tokens = 128000 row lookups) with the weight table replicated
per core in DRAM.

Per core the token stream is cut into 50 chunks of T=128 tokens (2560
lookup slots each).  For each chunk:
  1. Host splits the chunk's ids into 4 vocab ranges of <=32768 rows
     (range = id >> 15, local = id & 32767 — int16-safe for dma_gather)
     as compact per-(chunk,range) lists padded to 128-slot blocks (pad
     descriptors fetch row 0, so every gathered block is initialized).
  2. nc.gpsimd.dma_gather fetches the rows from a [V, 128] fp16 table in
     which each embedding row appears duplicated (row|row) so one 256 B
     descriptor (the SWDGE minimum) delivers the fp16 row.  The gathers
     for 10 consecutive chunks are MERGED into one call per range (the
     per-call Pool-serialized fixed cost, ~2.2 us, otherwise dominates)
     and split across the 4 SWDGE queues with exactly balanced descriptor
     counts: 4-queue issue improves descriptor generation+drain from
     ~8.9 ns/desc to ~1.4 ns/desc.
  3. One DVE tensor_tensor per (chunk, range) builds all of its fp16 0/1
     selection matrices S[p, tok] = is_equal(token_of_slot[p], iota[tok])
     via stride-0 broadcast APs (merging kills per-block DVE dispatch).
  4. The TensorEngine accumulates psum[64, 128] += cg_block[128, 64].T @
     S[128, 128] over the chunk's blocks (fp16, 1 cyc/row; fp32 would be
     4 cyc/row).  Padding slots carry token -1 and select nothing.
  5. Bias (a [64, 1] f32 column) is added on PSUM eviction; the [64, 128]
     f32 tile is DMA'd out and the host transposes back at the end.

Measured on HW: ~0.28 ms vs 1.20 ms for the f32/single-queue/per-chunk
baseline (same math, rel err 2.4e-4 from fp16 weights, tolerance 2e-2).
"""

import numpy as np

import concourse.bass as bass
import concourse.tile as tile
from concourse import bacc, mybir
from concourse.bass_utils import run_bass_kernel_spmd

B, S, K, E, V = 1024, 50, 20, 64, 100000
N_CORES = 8
P = 128
TOK_CORE = B // N_CORES * S          # 6400 tokens per core

T3 = 128                             # tokens per chunk (PSUM window)
CH3 = TOK_CORE // T3                 # 50 chunks per core
M_SC = 10                            # chunks per superchunk (gather merge)
N_SC = CH3 // M_SC                   # 5 superchunks
RANGE_BASES = [0, 32768, 65536, 98304]
RANGE_SIZES = [32768, 32768, 32768, V - 98304]
N_QUEUES = 4
DMA_SCRATCH = 32768

_cache: dict = {}


def default_n_slots(t=T3):
    m = t * K * 32768 / V
    sig = (t * K * 0.32768 * (1 - 0.32768)) ** 0.5
    pad = int(np.ceil((m + 6 * sig) / P) * P)
    mt = t * K * (V - 98304) / V
    sig_t = (t * K * (1 - 98304 / V) * 98304 / V) ** 0.5
    pad_t = int(np.ceil((mt + 6 * sig_t) / P) * P)
    return (pad, pad, pad, pad_t)


def _plan(n_slots, reg_counts, t=T3):
    """Static per-NEFF plan: per (sc, r) total blocks and per-chunk offsets."""
    ch = TOK_CORE // t
    nbc = [[reg_counts[c][r] // P for r in range(4)] for c in range(ch)]
    NB = [[sum(nbc[sc * M_SC + m][r] for m in range(M_SC)) for r in range(4)]
          for sc in range(N_SC)]
    cum = [[[0] * M_SC for _ in range(4)] for _ in range(N_SC)]
    for sc in range(N_SC):
        for r in range(4):
            acc = 0
            for m in range(M_SC):
                cum[sc][r][m] = acc
                acc += nbc[sc * M_SC + m][r]
    NBmax = [max(NB[sc][r] for sc in range(N_SC)) for r in range(4)]
    nw16 = [sum(NB[sc][r] for r in range(4)) * 8 for sc in range(N_SC)]
    return nbc, NB, cum, NBmax, max(nw16)


def _build_v5(n_slots, n_repeat=1, reg_counts=None, t=T3):
    ch = TOK_CORE // t
    nblk = tuple(n // P for n in n_slots)
    nblk_tot = sum(nblk)
    nbc, NB, cum, NBmax, nw16max = _plan(n_slots, reg_counts, t)

    nc = bacc.Bacc("TRN2", target_bir_lowering=False, debug=False,
                   num_devices=N_CORES,
                   num_swdge_queues=N_QUEUES,
                   dynamic_dma_scratch_size=DMA_SCRATCH)
    wt = nc.dram_tensor("wt", [V, 2 * E], mybir.dt.float16,
                        kind="ExternalInput")
    gidx = nc.dram_tensor("gidx", [N_SC, P, nw16max], mybir.dt.int16,
                          kind="ExternalInput")
    tokf = nc.dram_tensor("tokf", [N_SC, P, M_SC * nblk_tot],
                          mybir.dt.float16, kind="ExternalInput")
    iota = nc.dram_tensor("iota", [P, t], mybir.dt.float16,
                          kind="ExternalInput")
    biasc = nc.dram_tensor("biasc", [E, 1], mybir.dt.float32,
                           kind="ExternalInput")
    y = nc.dram_tensor("y", [ch, E, t], mybir.dt.float32,
                       kind="ExternalOutput")

    with tile.TileContext(nc) as tc:
        with (
            tc.tile_pool(name="idxp", bufs=2) as idxp,
            tc.tile_pool(name="constp", bufs=1) as constp,
            tc.tile_pool(name="cgp", bufs=2) as cgp,
            tc.tile_pool(name="sp", bufs=4) as sp,
            tc.tile_pool(name="psump", bufs=2, space="PSUM") as psump,
            tc.tile_pool(name="evp", bufs=3) as evp,
        ):
            iota_t = constp.tile([P, t], mybir.dt.float16)
            nc.sync.dma_start(out=iota_t[:, :], in_=iota[:, :])
            biasc_t = constp.tile([E, 1], mybir.dt.float32)
            nc.sync.dma_start(out=biasc_t[:, :], in_=biasc[:, :])

            for _ in range(n_repeat):
                for sc in range(N_SC):
                    gidx_t = idxp.tile([P, nw16max], mybir.dt.int16,
                                       tag="gidx")
                    nc.sync.dma_start(out=gidx_t[:, :], in_=gidx[sc])
                    tokf_t = idxp.tile([P, M_SC * nblk_tot],
                                       mybir.dt.float16, tag="tokf")
                    nc.sync.dma_start(out=tokf_t[:, :], in_=tokf[sc])

                    # balanced queue plan: split each range's block list so
                    # every queue generates ~totB/4 blocks of descriptors
                    totB = sum(NB[sc])
                    qcap = [totB // N_QUEUES + (1 if i < totB % N_QUEUES
                                               else 0)
                            for i in range(N_QUEUES)]
                    cgs = []
                    off = 0
                    cur_q = 0
                    for r in range(4):
                        nb_sc = NB[sc][r]
                        cg = cgp.tile([P, NBmax[r], 2 * E],
                                      mybir.dt.float16, tag=f"cg{r}")
                        start = 0
                        while start < nb_sc:
                            while qcap[cur_q] == 0:
                                cur_q += 1
                            take = min(nb_sc - start, qcap[cur_q])
                            qcap[cur_q] -= take
                            nc.gpsimd.dma_gather(
                                out_ap=cg[:, start:start + take, :],
                                in_ap=wt[RANGE_BASES[r]:
                                         RANGE_BASES[r] + RANGE_SIZES[r], :],
                                idxs_ap=gidx_t[:, off + start * 8:
                                               off + (start + take) * 8],
                                num_idxs=take * P,
                                num_idxs_reg=take * P,
                                elem_size=2 * E,
                                single_packet=False,
                                queue_num=cur_q,
                            )
                            start += take
                        cgs.append(cg)
                        off += nb_sc * 8

                    for m in range(M_SC):
                        c = sc * M_SC + m
                        s_ts = []
                        boff = 0
                        for r in range(4):
                            nb = nbc[c][r]
                            s_t = sp.tile([P, nb, t], mybir.dt.float16,
                                          tag=f"S{r}")
                            col = m * nblk_tot + boff
                            nc.vector.tensor_tensor(
                                out=s_t[:, :, :],
                                in0=tokf_t[:, col:col + nb]
                                    .unsqueeze(2).to_broadcast([P, nb, t]),
                                in1=iota_t[:, :]
                                    .unsqueeze(1).to_broadcast([P, nb, t]),
                                op=mybir.AluOpType.is_equal)
                            s_ts.append(s_t)
                            boff += nblk[r]

                        nbc_tot = sum(nbc[c])
                        psum = psump.tile([E, t], mybir.dt.float32, tag="ps")
                        blk = 0
                        for r in range(4):
                            for bb in range(nbc[c][r]):
                                nc.tensor.matmul(
                                    out=psum[:, :],
                                    lhsT=cgs[r][:, cum[sc][r][m] + bb, 0:E],
                                    rhs=s_ts[r][:, bb, :],
                                    start=(blk == 0),
                                    stop=(blk == nbc_tot - 1))
                                blk += 1

                        ev = evp.tile([E, t], mybir.dt.float32, tag="ev")
                        nc.vector.tensor_tensor(
                            out=ev[:, :], in0=psum[:, :],
                            in1=biasc_t[:, 0:1].to_broadcast([E, t]),
                            op=mybir.AluOpType.add)
                        nc.sync.dma_start(out=y[c], in_=ev[:, :])
    nc.compile()
    return nc


def _wrap16(flat: np.ndarray) -> np.ndarray:
    """int16 list -> [128, n/16] layout (index i at partition i%16, column
    i//16, replicated across the 8 16-partition Q7 groups)."""
    n = flat.shape[0]
    blk = flat.reshape(n // 16, 16).T
    return np.tile(blk, (8, 1))


def _build_indices_v5(ids_core, n_slots, reg_counts, t=T3):
    ch = TOK_CORE // t
    nblk = tuple(n // P for n in n_slots)
    nblk_tot = sum(nblk)
    nbc, NB, cum, NBmax, nw16max = _plan(n_slots, reg_counts, t)
    gidx = np.zeros((N_SC, P, nw16max), np.int16)
    tokf = np.zeros((N_SC, P, M_SC * nblk_tot), np.float16)
    tok_of_slot = np.arange(t * K) // K

    for sc in range(N_SC):
        off = 0
        for r in range(4):
            for m in range(M_SC):
                c = sc * M_SC + m
                flat = ids_core[c * t:(c + 1) * t].reshape(-1)
                rng_id = flat >> 15
                local = flat & 32767
                sel = np.nonzero(rng_id == r)[0]
                n_c = nbc[c][r] * P
                assert sel.shape[0] <= n_c
                g = np.zeros(n_c, np.int16)          # pads fetch row 0
                g[:sel.shape[0]] = local[sel]
                gidx[sc, :, off:off + n_c // 16] = _wrap16(g)
                off += n_c // 16
                tf = np.full(n_c, -1.0, np.float16)  # pads select no token
                tf[:sel.shape[0]] = tok_of_slot[sel]
                boff = m * nblk_tot + sum(nblk[:r])
                tokf[sc, :, boff:boff + nbc[c][r]] = \
                    tf.reshape(nbc[c][r], P).T
    return gidx, tokf


def _host_prep(content_input, W, b, t=T3):
    ids = np.ascontiguousarray(content_input).astype(np.int32).reshape(B * S, K)
    w16 = np.ascontiguousarray(W.T.astype(np.float16))          # [V, E]
    wt2 = np.ascontiguousarray(np.concatenate([w16, w16], axis=1))
    iota = np.ascontiguousarray(
        np.broadcast_to(np.arange(t, dtype=np.float16), (P, t)))
    biasc = np.ascontiguousarray(b.astype(np.float32).reshape(E, 1))

    ch = TOK_CORE // t
    per_core = [ids[i * TOK_CORE:(i + 1) * TOK_CORE] for i in range(N_CORES)]
    # per-(chunk, range) max count across cores, rounded up to 128-blocks;
    # static list sizes grow (one recompile) only if an input distribution
    # overflows the default padding
    cnt = np.zeros((ch, 4), np.int64)
    for pc in per_core:
        for c in range(ch):
            r = pc[c * t:(c + 1) * t].reshape(-1) >> 15
            cnt[c] = np.maximum(cnt[c], np.bincount(r, minlength=4))
    dflt = default_n_slots(t)
    n_slots = tuple(
        int(max(d, -(-int(m) // P) * P))
        for d, m in zip(dflt, cnt.max(axis=0)))
    reg_counts = tuple(
        tuple(int(-(-max(int(v), 1) // P) * P) for v in row)
        for row in cnt)
    return wt2, iota, biasc, per_core, n_slots, reg_counts


def make_in_maps(content_input, W, b, t=T3):
    wt2, iota, biasc, per_core, n_slots, reg_counts = _host_prep(
        content_input, W, b, t)
    in_maps = []
    for i in range(N_CORES):
        gidx, tokf = _build_indices_v5(per_core[i], n_slots, reg_counts, t)
        in_maps.append({"wt": wt2, "gidx": gidx, "tokf": tokf,
                        "iota": iota, "biasc": biasc})
    return in_maps, n_slots, reg_counts


def kernel(content_input: np.ndarray, W: np.ndarray, b: np.ndarray) -> np.ndarray:
    in_maps, n_slots, reg_counts = make_in_maps(content_input, W, b)
    key = ("nc5", T3, n_slots, reg_counts)
    if key not in _cache:
        _cache[key] = _build_v5(n_slots, reg_counts=reg_counts)
    nc = _cache[key]
    res = run_bass_kernel_spmd(nc, in_maps, core_ids=list(range(N_CORES)))
    # y[c, :, t] holds out[token c*T3 + t, :] transposed
    out = np.concatenate(
        [res.results[i]["y"].transpose(0, 2, 1).reshape(TOK_CORE, E)
         for i in range(N_CORES)],
        axis=0)
    return out.reshape(B, S, E)
